# revision 4
# baseline (speedup 1.0000x reference)
"""MoE layer (top-2 of 8 experts) on 8 Trainium2 NeuronCores.

Strategy: data-parallel over tokens (8192 tokens -> 1024/core), dense
expert compute on-device (router, top-k, softmax, combine, both expert
matmuls, weighted combine all run on the NeuronCore). Host only splits /
transposes / concatenates.

Self-contained: hardcodes all shapes from the problem spec.
"""

import numpy as np

import concourse.bass as bass
import concourse.mybir as mybir
import concourse.tile as tile
from concourse.bass_utils import run_bass_kernel_spmd

# Problem dims
B, T, D, H, E, TOPK = 4, 2048, 512, 1024, 8, 2
N_CORES = 8
N = B * T                     # 8192 tokens
NPC = N // N_CORES            # 1024 tokens per core
P = 128                       # partitions
D_CH = D // P                 # 4 contraction chunks for D
H_CH = H // P                 # 8 contraction chunks for H
TOK_TILES = NPC // P          # 8 token tiles per core
NB = 2                        # token free-dim chunks of 512
NBW = NPC // NB               # 512
TPB = TOK_TILES // NB         # token tiles per free-dim chunk

F32 = mybir.dt.float32
F32R = mybir.dt.float32r
I32 = mybir.dt.int32
U32 = mybir.dt.uint32

# CTRL-class instructions (NoOp/Drain/...) accept only ONE sync-wait
# command in this walrus build; others accept two. Peel excess waits
# onto preceding NOPs (waiting A,B then C == waiting on A,B,C).
_CTRL_TYPES = ("InstNoOp", "InstDrain", "InstEventSemaphore", "InstAllEngineBarrier")


def _split_waits(nc):
    for f in nc.m.functions:
        for bb in f.blocks:
            insts = list(bb.instructions)
            out = []
            changed = False
            for ins in insts:
                si = ins.sync_info
                cap = 1
                if si is not None and si.on_wait and len(si.on_wait) > cap:
                    waits = list(si.on_wait)
                    k = 0
                    while len(waits) > cap:
                        nop = mybir.InstNoOp(
                            name=f"{ins.name}-waitsplit-{k}",
                            engine=ins.engine,
                            sync_info=mybir.SyncInfo(on_wait=waits[:1], on_update=[]),
                            bass_nofuse=True,
                        )
                        waits = waits[1:]
                        out.append(nop)
                        k += 1
                    si.on_wait = waits
                    changed = True
                out.append(ins)
            if changed:
                bb.instructions.clear()
                for ins in out:
                    bb.add_instruction(ins)


def build_nc(use_f32r=True, split_waits=True):
    """Build the per-core Bass program (same SPMD program on all cores)."""
    nc = bass.Bass()

    xT_d = nc.declare_dram_parameter("xT", [D, NPC], F32, isOutput=False)
    Wg_d = nc.declare_dram_parameter("Wg", [D, E], F32, isOutput=False)
    W1_d = nc.declare_dram_parameter("W1", [E, D, H], F32, isOutput=False)
    b1_d = nc.declare_dram_parameter("b1", [E, H], F32, isOutput=False)
    W2_d = nc.declare_dram_parameter("W2", [E, H, D], F32, isOutput=False)
    b2_d = nc.declare_dram_parameter("b2", [E, D], F32, isOutput=False)
    id_d = nc.declare_dram_parameter("ident", [P, P], F32, isOutput=False)

    out_d = nc.declare_dram_parameter("out", [NPC, D], F32, isOutput=True)
    alog_d = nc.declare_dram_parameter("aux_logits", [NPC, E], F32, isOutput=True)
    aidx_d = nc.declare_dram_parameter("aux_idx", [NPC, TOPK], I32, isOutput=True)
    aw_d = nc.declare_dram_parameter("aux_w", [NPC, TOPK], F32, isOutput=True)

    def mmdt(ap):
        return ap.bitcast(F32R) if use_f32r else ap

    with tile.TileContext(nc) as tc:
        with (
            tc.tile_pool(name="persist", bufs=1) as pp,
            tc.tile_pool(name="router", bufs=2) as rp,
            tc.tile_pool(name="w1p", bufs=2) as w1p,
            tc.tile_pool(name="w2p", bufs=2) as w2p,
            tc.tile_pool(name="htp", bufs=2) as htp,
        ):
            # ---- resident loads ----
            xt = []
            for kc in range(D_CH):
                t_ = pp.tile([P, NPC], F32, tag=f"xt{kc}")
                nc.sync.dma_start(t_[:], xT_d[kc * P:(kc + 1) * P, :])
                xt.append(t_)
            wg = pp.tile([P, D_CH, E], F32, tag="wg")
            for kc in range(D_CH):
                nc.sync.dma_start(wg[:, kc, :], Wg_d[kc * P:(kc + 1) * P, :])
            b1_all = pp.tile([P, E, H_CH], F32, tag="b1")
            for e in range(E):
                nc.sync.dma_start(
                    b1_all[:, e, :], b1_d[e].rearrange("(hc p) -> p hc", p=P)
                )
            b2row = pp.tile([E, D], F32, tag="b2")
            nc.sync.dma_start(b2row[:], b2_d[:])
            ident = pp.tile([P, P], F32, tag="ident")
            nc.sync.dma_start(ident[:], id_d[:])

            out_acc = [pp.tile([P, D], F32, tag=f"oacc{t}", name=f"oacc{t}")
                       for t in range(TOK_TILES)]
            c_sb = [pp.tile([P, E], F32, tag=f"comb{t}", name=f"comb{t}")
                    for t in range(TOK_TILES)]

            # ---- phase 1: router, top-2, softmax, combine ----
            with tc.tile_pool(name="rpsum", bufs=2, space="PSUM") as rpsum_p:
                for t in range(TOK_TILES):
                    tok = slice(t * P, (t + 1) * P)
                    rpsum = rpsum_p.tile([P, E], F32)
                    for kc in range(D_CH):
                        nc.tensor.matmul(
                            rpsum[:],
                            xt[kc][:, tok],
                            wg[:, kc, :],
                            start=(kc == 0),
                            stop=(kc == D_CH - 1),
                        )
                    r_sb = rp.tile([P, E], F32, tag="r")
                    nc.vector.tensor_copy(r_sb[:], rpsum[:])
                    nc.sync.dma_start(alog_d[tok, :], r_sb[:])

                    mx = rp.tile([P, 8], F32, tag="mx")
                    ix = rp.tile([P, 8], U32, tag="ix")
                    nc.vector.max_with_indices(mx[:], ix[:], r_sb[:])
                    ix32 = rp.tile([P, TOPK], I32, tag="ix32")
                    nc.vector.tensor_copy(ix32[:], ix[:, 0:TOPK])
                    nc.sync.dma_start(aidx_d[tok, :], ix32[:])

                    l1 = mx[:, 0:1]
                    l2 = mx[:, 1:2]
                    dlt = rp.tile([P, 1], F32, tag="dlt")
                    nc.vector.tensor_sub(dlt[:], l2, l1)
                    e2 = rp.tile([P, 1], F32, tag="e2")
                    nc.scalar.activation(e2[:], dlt[:], mybir.ActivationFunctionType.Exp)
                    den = rp.tile([P, 1], F32, tag="den")
                    nc.vector.tensor_scalar_add(den[:], e2[:], 1.0)
                    w1_ = rp.tile([P, 1], F32, tag="w1")
                    nc.vector.reciprocal(w1_[:], den[:])
                    w2_ = rp.tile([P, 1], F32, tag="w2")
                    nc.vector.tensor_mul(w2_[:], e2[:], w1_[:])

                    aw = rp.tile([P, TOPK], F32, tag="aw")
                    nc.vector.tensor_copy(aw[:, 0:1], w1_[:])
                    nc.vector.tensor_copy(aw[:, 1:2], w2_[:])
                    nc.sync.dma_start(aw_d[tok, :], aw[:])

                    # combine[t] = (r == l1) * w1 + (r == l2) * w2
                    eq = rp.tile([P, E], F32, tag="eq")
                    nc.vector.tensor_scalar(
                        c_sb[t][:], r_sb[:], l1, None, op0=mybir.AluOpType.is_equal
                    )
                    nc.vector.tensor_scalar(
                        c_sb[t][:], c_sb[t][:], w1_[:], None, op0=mybir.AluOpType.mult
                    )
                    nc.vector.tensor_scalar(
                        eq[:], r_sb[:], l2, None, op0=mybir.AluOpType.is_equal
                    )
                    nc.vector.scalar_tensor_tensor(
                        c_sb[t][:],
                        eq[:],
                        w2_[:],
                        c_sb[t][:],
                        op0=mybir.AluOpType.mult,
                        op1=mybir.AluOpType.add,
                    )

            # ---- phase 2: out_acc[t] = combine[t] @ b2 (exact b2 handling) ----
            with (
                tc.tile_pool(name="ctpsum", bufs=2, space="PSUM") as ctp,
                tc.tile_pool(name="zpsum", bufs=2, space="PSUM") as zp,
            ):
                for t in range(TOK_TILES):
                    ct_ps = ctp.tile([E, P], F32)
                    nc.tensor.transpose(ct_ps[:], c_sb[t][:], ident[:])
                    ct_sb = rp.tile([E, P], F32, tag="ctsb")
                    nc.vector.tensor_copy(ct_sb[:], ct_ps[:])
                    z_ps = zp.tile([P, D], F32)
                    nc.tensor.matmul(z_ps[:], ct_sb[:], b2row[:], start=True, stop=True)
                    nc.vector.tensor_copy(out_acc[t][:], z_ps[:])

            # ---- phase 3: experts ----
            with (
                tc.tile_pool(name="hpsum", bufs=3, space="PSUM") as hp,
                tc.tile_pool(name="ypsum", bufs=3, space="PSUM") as yp,
            ):
                for e in range(E):
                    w1t = w1p.tile([P, D_CH, H], F32, tag="w1t")
                    for kc in range(D_CH):
                        nc.sync.dma_start(
                            w1t[:, kc, :], W1_d[e, kc * P:(kc + 1) * P, :]
                        )
                    w2t = w2p.tile([P, H_CH, D], F32, tag="w2t")
                    for hc in range(H_CH):
                        nc.sync.dma_start(
                            w2t[:, hc, :], W2_d[e, hc * P:(hc + 1) * P, :]
                        )

                    for nb in range(NB):
                        cols = slice(nb * NBW, (nb + 1) * NBW)
                        hts = htp.tile([P, H_CH, NBW], F32, tag="hts")
                        for hc in range(H_CH):
                            hpsum = hp.tile([P, NBW], F32)
                            for kc in range(D_CH):
                                nc.tensor.matmul(
                                    hpsum[:],
                                    mmdt(w1t[:, kc, hc * P:(hc + 1) * P]),
                                    mmdt(xt[kc][:, cols]),
                                    start=(kc == 0),
                                    stop=(kc == D_CH - 1),
                                )
                            nc.scalar.activation(
                                hts[:, hc, :],
                                hpsum[:],
                                mybir.ActivationFunctionType.Gelu,
                                bias=b1_all[:, e, hc:hc + 1],
                                scale=1.0,
                            )
                        for tt in range(TPB):
                            t = nb * TPB + tt
                            ypsum = yp.tile([P, D], F32)
                            for hc in range(H_CH):
                                nc.tensor.matmul(
                                    ypsum[:],
                                    mmdt(hts[:, hc, tt * P:(tt + 1) * P]),
                                    mmdt(w2t[:, hc, :]),
                                    start=(hc == 0),
                                    stop=(hc == H_CH - 1),
                                )
                            nc.vector.scalar_tensor_tensor(
                                out_acc[t][:],
                                ypsum[:],
                                c_sb[t][:, e:e + 1],
                                out_acc[t][:],
                                op0=mybir.AluOpType.mult,
                                op1=mybir.AluOpType.add,
                            )

            # ---- phase 4: store ----
            for t in range(TOK_TILES):
                nc.sync.dma_start(out_d[t * P:(t + 1) * P, :], out_acc[t][:])

    if split_waits:
        _split_waits(nc)
    return nc


_NC_CACHE = {}


def _get_nc(use_f32r=True):
    if use_f32r not in _NC_CACHE:
        _NC_CACHE[use_f32r] = build_nc(use_f32r)
    return _NC_CACHE[use_f32r]


def make_in_maps(x, Wg, W1, b1, W2, b2):
    xf = np.ascontiguousarray(np.asarray(x, dtype=np.float32).reshape(N, D))
    Wg = np.ascontiguousarray(np.asarray(Wg, dtype=np.float32))
    W1 = np.ascontiguousarray(np.asarray(W1, dtype=np.float32))
    b1 = np.ascontiguousarray(np.asarray(b1, dtype=np.float32))
    W2 = np.ascontiguousarray(np.asarray(W2, dtype=np.float32))
    b2 = np.ascontiguousarray(np.asarray(b2, dtype=np.float32))
    ident = np.eye(P, dtype=np.float32)
    in_maps = []
    for c in range(N_CORES):
        xT = np.ascontiguousarray(xf[c * NPC:(c + 1) * NPC].T)
        in_maps.append(
            {"xT": xT, "Wg": Wg, "W1": W1, "b1": b1, "W2": W2, "b2": b2,
             "ident": ident}
        )
    return in_maps


def assemble(results):
    out = np.concatenate([results[c]["out"] for c in range(N_CORES)], axis=0)
    alog = np.concatenate([results[c]["aux_logits"] for c in range(N_CORES)], axis=0)
    aidx = np.concatenate([results[c]["aux_idx"] for c in range(N_CORES)], axis=0)
    aw = np.concatenate([results[c]["aux_w"] for c in range(N_CORES)], axis=0)
    return (
        out.reshape(B, T, D).astype(np.float32),
        alog.reshape(B, T, E).astype(np.float32),
        aidx.reshape(B, T, TOPK).astype(np.int32),
        aw.reshape(B, T, TOPK).astype(np.float32),
    )


def kernel(x, Wg, W1, b1, W2, b2):
    nc = _get_nc()
    in_maps = make_in_maps(x, Wg, W1, b1, W2, b2)
    res = run_bass_kernel_spmd(nc, in_maps, list(range(N_CORES)))
    return assemble(res.results)


# revision 6
# speedup vs baseline: 3.0806x; 3.0806x over previous
"""MoE layer (top-2 of 8 experts) on 8 Trainium2 NeuronCores.

Strategy: data-parallel over tokens (8192 tokens -> 1024/core), dense
expert compute on-device (router, top-k, softmax, combine, both expert
matmuls, weighted combine all run on the NeuronCore). Host only splits /
transposes / concatenates.

Self-contained: hardcodes all shapes from the problem spec.
"""

import numpy as np

import concourse.bass as bass
import concourse.mybir as mybir
import concourse.tile as tile
from concourse.bass_utils import run_bass_kernel_spmd

# Problem dims
B, T, D, H, E, TOPK = 4, 2048, 512, 1024, 8, 2
N_CORES = 8
N = B * T                     # 8192 tokens
NPC = N // N_CORES            # 1024 tokens per core
P = 128                       # partitions
D_CH = D // P                 # 4 contraction chunks for D
H_CH = H // P                 # 8 contraction chunks for H
TOK_TILES = NPC // P          # 8 token tiles per core
NB = 2                        # token free-dim chunks of 512
NBW = NPC // NB               # 512
TPB = TOK_TILES // NB         # token tiles per free-dim chunk

F32 = mybir.dt.float32
F32R = mybir.dt.float32r
I32 = mybir.dt.int32
U32 = mybir.dt.uint32

# CTRL-class instructions (NoOp/Drain/...) accept only ONE sync-wait
# command in this walrus build; others accept two. Peel excess waits
# onto preceding NOPs (waiting A,B then C == waiting on A,B,C).
_CTRL_TYPES = ("InstNoOp", "InstDrain", "InstEventSemaphore", "InstAllEngineBarrier")


def _split_waits(nc):
    for f in nc.m.functions:
        for bb in f.blocks:
            insts = list(bb.instructions)
            out = []
            changed = False
            for ins in insts:
                si = ins.sync_info
                cap = 1
                if si is not None and si.on_wait and len(si.on_wait) > cap:
                    waits = list(si.on_wait)
                    k = 0
                    while len(waits) > cap:
                        nop = mybir.InstNoOp(
                            name=f"{ins.name}-waitsplit-{k}",
                            engine=ins.engine,
                            sync_info=mybir.SyncInfo(on_wait=waits[:1], on_update=[]),
                            bass_nofuse=True,
                        )
                        waits = waits[1:]
                        out.append(nop)
                        k += 1
                    si.on_wait = waits
                    changed = True
                out.append(ins)
            if changed:
                bb.instructions.clear()
                for ins in out:
                    bb.add_instruction(ins)


def build_nc(use_f32r=True, split_waits=True):
    """Build the per-core Bass program (same SPMD program on all cores)."""
    nc = bass.Bass()

    MMDT = F32R if use_f32r else F32
    xT_d = nc.declare_dram_parameter("xT", [D, NPC], MMDT, isOutput=False)
    xTf_d = (nc.declare_dram_parameter("xTf", [D, NPC], F32, isOutput=False)
             if use_f32r else xT_d)
    Wg_d = nc.declare_dram_parameter("Wg", [D, E], F32, isOutput=False)
    W1_d = nc.declare_dram_parameter("W1", [E, D, H], MMDT, isOutput=False)
    b1_d = nc.declare_dram_parameter("b1", [E, H], F32, isOutput=False)
    W2_d = nc.declare_dram_parameter("W2", [E, H, D], MMDT, isOutput=False)
    b2_d = nc.declare_dram_parameter("b2", [E, D], F32, isOutput=False)
    id_d = nc.declare_dram_parameter("ident", [P, P], F32, isOutput=False)

    out_d = nc.declare_dram_parameter("out", [NPC, D], F32, isOutput=True)
    alog_d = nc.declare_dram_parameter("aux_logits", [NPC, E], F32, isOutput=True)
    aidx_d = nc.declare_dram_parameter("aux_idx", [NPC, TOPK], I32, isOutput=True)
    aw_d = nc.declare_dram_parameter("aux_w", [NPC, TOPK], F32, isOutput=True)


    with tile.TileContext(nc) as tc:
        with (
            tc.tile_pool(name="persist", bufs=1) as pp,
            tc.tile_pool(name="router", bufs=2) as rp,
            tc.tile_pool(name="w1p", bufs=2) as w1p,
            tc.tile_pool(name="w2p", bufs=2) as w2p,
            tc.tile_pool(name="htp", bufs=2) as htp,
        ):
            # ---- resident loads ----
            xt = []
            for kc in range(D_CH):
                t_ = pp.tile([P, NPC], MMDT, tag=f"xt{kc}")
                nc.sync.dma_start(t_[:], xT_d[kc * P:(kc + 1) * P, :])
                xt.append(t_)
            if use_f32r:
                xtf = []
                for kc in range(D_CH):
                    tf_ = pp.tile([P, NPC], F32, tag=f"xtf{kc}", name=f"xtf{kc}")
                    nc.sync.dma_start(tf_[:], xTf_d[kc * P:(kc + 1) * P, :])
                    xtf.append(tf_)
            else:
                xtf = xt
            wg = pp.tile([P, D_CH, E], F32, tag="wg")
            for kc in range(D_CH):
                nc.sync.dma_start(wg[:, kc, :], Wg_d[kc * P:(kc + 1) * P, :])
            b1_all = pp.tile([P, E, H_CH], F32, tag="b1")
            for e in range(E):
                nc.sync.dma_start(
                    b1_all[:, e, :], b1_d[e].rearrange("(hc p) -> p hc", p=P)
                )
            b2row = pp.tile([E, D], F32, tag="b2")
            nc.sync.dma_start(b2row[:], b2_d[:])
            ident = pp.tile([P, P], F32, tag="ident")
            nc.sync.dma_start(ident[:], id_d[:])

            out_acc = [pp.tile([P, D], F32, tag=f"oacc{t}", name=f"oacc{t}")
                       for t in range(TOK_TILES)]
            c_sb = [pp.tile([P, E], F32, tag=f"comb{t}", name=f"comb{t}")
                    for t in range(TOK_TILES)]

            # ---- phase 1: router, top-2, softmax, combine ----
            with tc.tile_pool(name="rpsum", bufs=2, space="PSUM") as rpsum_p:
                for t in range(TOK_TILES):
                    tok = slice(t * P, (t + 1) * P)
                    rpsum = rpsum_p.tile([P, E], F32)
                    for kc in range(D_CH):
                        nc.tensor.matmul(
                            rpsum[:],
                            xtf[kc][:, tok].bitcast(F32),
                            wg[:, kc, :],
                            start=(kc == 0),
                            stop=(kc == D_CH - 1),
                        )
                    r_sb = rp.tile([P, E], F32, tag="r")
                    nc.vector.tensor_copy(r_sb[:], rpsum[:])
                    nc.sync.dma_start(alog_d[tok, :], r_sb[:])

                    mx = rp.tile([P, 8], F32, tag="mx")
                    ix = rp.tile([P, 8], U32, tag="ix")
                    nc.vector.max_with_indices(mx[:], ix[:], r_sb[:])
                    ix32 = rp.tile([P, TOPK], I32, tag="ix32")
                    nc.vector.tensor_copy(ix32[:], ix[:, 0:TOPK])
                    nc.sync.dma_start(aidx_d[tok, :], ix32[:])

                    l1 = mx[:, 0:1]
                    l2 = mx[:, 1:2]
                    dlt = rp.tile([P, 1], F32, tag="dlt")
                    nc.vector.tensor_sub(dlt[:], l2, l1)
                    e2 = rp.tile([P, 1], F32, tag="e2")
                    nc.scalar.activation(e2[:], dlt[:], mybir.ActivationFunctionType.Exp)
                    den = rp.tile([P, 1], F32, tag="den")
                    nc.vector.tensor_scalar_add(den[:], e2[:], 1.0)
                    w1_ = rp.tile([P, 1], F32, tag="w1")
                    nc.vector.reciprocal(w1_[:], den[:])
                    w2_ = rp.tile([P, 1], F32, tag="w2")
                    nc.vector.tensor_mul(w2_[:], e2[:], w1_[:])

                    aw = rp.tile([P, TOPK], F32, tag="aw")
                    nc.vector.tensor_copy(aw[:, 0:1], w1_[:])
                    nc.vector.tensor_copy(aw[:, 1:2], w2_[:])
                    nc.sync.dma_start(aw_d[tok, :], aw[:])

                    # combine[t] = (r == l1) * w1 + (r == l2) * w2
                    eq = rp.tile([P, E], F32, tag="eq")
                    nc.vector.tensor_scalar(
                        c_sb[t][:], r_sb[:], l1, None, op0=mybir.AluOpType.is_equal
                    )
                    nc.vector.tensor_scalar(
                        c_sb[t][:], c_sb[t][:], w1_[:], None, op0=mybir.AluOpType.mult
                    )
                    nc.vector.tensor_scalar(
                        eq[:], r_sb[:], l2, None, op0=mybir.AluOpType.is_equal
                    )
                    nc.vector.scalar_tensor_tensor(
                        c_sb[t][:],
                        eq[:],
                        w2_[:],
                        c_sb[t][:],
                        op0=mybir.AluOpType.mult,
                        op1=mybir.AluOpType.add,
                    )

            # ---- phase 2: out_acc[t] = combine[t] @ b2 (exact b2 handling) ----
            with (
                tc.tile_pool(name="ctpsum", bufs=2, space="PSUM") as ctp,
                tc.tile_pool(name="zpsum", bufs=2, space="PSUM") as zp,
            ):
                for t in range(TOK_TILES):
                    ct_ps = ctp.tile([E, P], F32)
                    nc.tensor.transpose(ct_ps[:], c_sb[t][:], ident[:])
                    ct_sb = rp.tile([E, P], F32, tag="ctsb")
                    nc.vector.tensor_copy(ct_sb[:], ct_ps[:])
                    z_ps = zp.tile([P, D], F32)
                    nc.tensor.matmul(z_ps[:], ct_sb[:], b2row[:], start=True, stop=True)
                    nc.vector.tensor_copy(out_acc[t][:], z_ps[:])

            # ---- phase 3: experts ----
            with (
                tc.tile_pool(name="hpsum", bufs=3, space="PSUM") as hp,
                tc.tile_pool(name="ypsum", bufs=3, space="PSUM") as yp,
            ):
                for e in range(E):
                    w1t = w1p.tile([P, D_CH, H], MMDT, tag="w1t")
                    for kc in range(D_CH):
                        nc.sync.dma_start(
                            w1t[:, kc, :], W1_d[e, kc * P:(kc + 1) * P, :]
                        )
                    w2t = w2p.tile([P, H_CH, D], MMDT, tag="w2t")
                    for hc in range(H_CH):
                        nc.sync.dma_start(
                            w2t[:, hc, :], W2_d[e, hc * P:(hc + 1) * P, :]
                        )

                    for nb in range(NB):
                        cols = slice(nb * NBW, (nb + 1) * NBW)
                        hts = htp.tile([P, H_CH, NBW], MMDT, tag="hts")
                        for hc in range(H_CH):
                            hpsum = hp.tile([P, NBW], F32)
                            for kc in range(D_CH):
                                nc.tensor.matmul(
                                    hpsum[:],
                                    w1t[:, kc, hc * P:(hc + 1) * P],
                                    xt[kc][:, cols],
                                    start=(kc == 0),
                                    stop=(kc == D_CH - 1),
                                )
                            nc.scalar.activation(
                                hts[:, hc, :],
                                hpsum[:],
                                mybir.ActivationFunctionType.Gelu,
                                bias=b1_all[:, e, hc:hc + 1],
                                scale=1.0,
                            )
                        for tt in range(TPB):
                            t = nb * TPB + tt
                            ypsum = yp.tile([P, D], F32)
                            for hc in range(H_CH):
                                nc.tensor.matmul(
                                    ypsum[:],
                                    hts[:, hc, tt * P:(tt + 1) * P],
                                    w2t[:, hc, :],
                                    start=(hc == 0),
                                    stop=(hc == H_CH - 1),
                                )
                            nc.vector.scalar_tensor_tensor(
                                out_acc[t][:],
                                ypsum[:],
                                c_sb[t][:, e:e + 1],
                                out_acc[t][:],
                                op0=mybir.AluOpType.mult,
                                op1=mybir.AluOpType.add,
                            )

            # ---- phase 4: store ----
            for t in range(TOK_TILES):
                nc.sync.dma_start(out_d[t * P:(t + 1) * P, :], out_acc[t][:])

    if split_waits:
        _split_waits(nc)
    return nc


_NC_CACHE = {}


def _get_nc(use_f32r=True):
    if use_f32r not in _NC_CACHE:
        _NC_CACHE[use_f32r] = build_nc(use_f32r)
    return _NC_CACHE[use_f32r]


def make_in_maps(x, Wg, W1, b1, W2, b2):
    xf = np.ascontiguousarray(np.asarray(x, dtype=np.float32).reshape(N, D))
    Wg = np.ascontiguousarray(np.asarray(Wg, dtype=np.float32))
    W1 = np.ascontiguousarray(np.asarray(W1, dtype=np.float32))
    b1 = np.ascontiguousarray(np.asarray(b1, dtype=np.float32))
    W2 = np.ascontiguousarray(np.asarray(W2, dtype=np.float32))
    b2 = np.ascontiguousarray(np.asarray(b2, dtype=np.float32))
    ident = np.eye(P, dtype=np.float32)
    in_maps = []
    for c in range(N_CORES):
        xT = np.ascontiguousarray(xf[c * NPC:(c + 1) * NPC].T)
        in_maps.append(
            {"xT": xT, "xTf": xT, "Wg": Wg, "W1": W1, "b1": b1, "W2": W2,
             "b2": b2, "ident": ident}
        )
    return in_maps


def assemble(results):
    out = np.concatenate([results[c]["out"] for c in range(N_CORES)], axis=0)
    alog = np.concatenate([results[c]["aux_logits"] for c in range(N_CORES)], axis=0)
    aidx = np.concatenate([results[c]["aux_idx"] for c in range(N_CORES)], axis=0)
    aw = np.concatenate([results[c]["aux_w"] for c in range(N_CORES)], axis=0)
    return (
        out.reshape(B, T, D).astype(np.float32),
        alog.reshape(B, T, E).astype(np.float32),
        aidx.reshape(B, T, TOPK).astype(np.int32),
        aw.reshape(B, T, TOPK).astype(np.float32),
    )


def kernel(x, Wg, W1, b1, W2, b2):
    nc = _get_nc()
    in_maps = make_in_maps(x, Wg, W1, b1, W2, b2)
    res = run_bass_kernel_spmd(nc, in_maps, list(range(N_CORES)))
    return assemble(res.results)
